# revision 17
# baseline (speedup 1.0000x reference)
"""CRF forward (partition function) kernel for Trainium2, 8 NeuronCores.

Algorithm (exp-space linear recurrence, data-parallel over batch):
  alpha_{k+1} = feat_k + log(W_log-matmul alpha_k)  is rewritten as
  q_{k+1} = ef'_k * (W @ q_k)   with W[next,prev] = exp(trans[next,prev]),
  ef'_k = exp(feat_k - max_tag feat_k) (host prescale) * r_e (periodic
  renormalization by 1/colsum, computed on device every NK steps).
  The valid-length mask only freezes alpha at t = length[b] (mask is a
  prefix), so instead of masking on device we snapshot every state
  S_k for k in [S/2, S] to DRAM and the host picks S_{length[b]}.

Layout per core (128 batch elems): "brick" = (groups of 32 tags stacked on
partitions, 32 batch elems on the free dim). Per chain of P=128/G partitions:
one bf16 matmul (block-diag W, PE) + one multiply (DVE, psum x ef -> bf16)
per step. G independent chains hide the PE<->DVE semaphore latency.

Host reconstructs: out[b] = log(q_L . exp(trans[END])) + cumsum(prescale)
                           + sum(log colsum_e applied before L).
"""

import os
import sys

import numpy as np
import ml_dtypes

if "/opt/trn_rl_repo" not in sys.path:
    sys.path.insert(0, "/opt/trn_rl_repo")

import concourse.bass as bass
import concourse.tile as tile
from concourse import bacc, mybir
from concourse.bass_utils import run_bass_kernel_spmd

BF = ml_dtypes.bfloat16
S, B, T = 1024, 1024, 32
START, END = T - 2, T - 1
NCORES = 8
BC = B // NCORES            # batch per core (128)
G = int(os.environ.get("CRF_G", "2"))  # independent chains per core
NK, EV0, LAG = 16, 4, 3      # renorm cadence / first event / apply lag
CHUNK = 128                 # ef steps per DMA chunk

dt = mybir.dt


def build_program(s_len=S, g=G):
    """Build the Bass program (one SPMD program for all cores)."""
    P = 128 // g            # partitions per chain
    NGRP = P // 32          # tag-groups per chain
    RING = s_len // 4       # ring slots per tile (2 tiles -> S/2 snapshots)
    chunk = min(CHUNK, s_len)
    n_ev = max(0, (s_len - EV0 - 1)) // NK + 1 if s_len > EV0 else 0

    nc = bacc.Bacc("TRN2", target_bir_lowering=False, num_devices=NCORES)

    ef_d = nc.dram_tensor("ef", [128, s_len * 32], dt.bfloat16, kind="ExternalInput")
    qi_d = nc.dram_tensor("qinit", [128, 32], dt.bfloat16, kind="ExternalInput")
    w_d = nc.dram_tensor("wblk", [P, P], dt.bfloat16, kind="ExternalInput")
    ob_d = nc.dram_tensor("onesblk", [P, NGRP], dt.bfloat16, kind="ExternalInput")
    oc_d = nc.dram_tensor("onesbc", [NGRP, P], dt.bfloat16, kind="ExternalInput")

    snapA = [nc.dram_tensor(f"snapsA_{c}", [P, RING * 32], dt.bfloat16,
                            kind="ExternalOutput") for c in range(g)]
    snapB = [nc.dram_tensor(f"snapsB_{c}", [P, RING * 32], dt.bfloat16,
                            kind="ExternalOutput") for c in range(g)]
    snapC = [nc.dram_tensor(f"snapsC_{c}", [P, 32], dt.bfloat16,
                            kind="ExternalOutput") for c in range(g)]
    rdump = [nc.dram_tensor(f"rdump_{c}", [NGRP, max(1, n_ev) * 32], dt.bfloat16,
                            kind="ExternalOutput") for c in range(g)]

    with tile.TileContext(nc) as tc:
        with (
            tc.tile_pool(name="singles", bufs=1) as singles,
            tc.tile_pool(name="efpool", bufs=2) as efpool,
            tc.tile_pool(name="efx", bufs=2) as efxpool,
            tc.tile_pool(name="psB", bufs=2, space="PSUM") as psb_pool,
            tc.tile_pool(name="psE", bufs=2, space="PSUM") as pse_pool,
        ):
            w_t = singles.tile([P, P], dt.bfloat16, tag="w", name="w_t")
            ob_t = singles.tile([P, NGRP], dt.bfloat16, tag="ob", name="ob_t")
            oc_t = singles.tile([NGRP, P], dt.bfloat16, tag="oc", name="oc_t")
            nc.sync.dma_start(out=w_t, in_=w_d.ap())
            nc.sync.dma_start(out=ob_t, in_=ob_d.ap())
            nc.sync.dma_start(out=oc_t, in_=oc_d.ap())

            rings = []   # [chain][0|1] -> persistent ring tile
            rbufs = []
            for c in range(g):
                rings.append([singles.tile([P, RING * 32], dt.bfloat16,
                                           tag=f"ring{c}_{h}",
                                           name=f"ring{c}_{h}") for h in range(2)])
                rbufs.append(singles.tile([NGRP, max(1, n_ev) * 32], dt.bfloat16,
                                          tag=f"rbuf{c}", name=f"rbuf{c}"))
                nc.sync.dma_start(out=rings[c][0][:, 0:32],
                                  in_=qi_d.ap()[c * P:(c + 1) * P, :])

            chunk_tiles = [[None, None] for _ in range(g)]  # double buffer slots
            pend = [{} for _ in range(g)]                   # k_apply -> psR tile

            n_chunks = (s_len + chunk - 1) // chunk
            for ch in range(n_chunks):
                for c in range(g):
                    t = efpool.tile([P, chunk * 32], dt.bfloat16, tag=f"efc{c}", name=f"efc{c}_{ch}")
                    nc.sync.dma_start(
                        out=t,
                        in_=ef_d.ap()[c * P:(c + 1) * P,
                                      ch * chunk * 32:(ch + 1) * chunk * 32])
                    chunk_tiles[c][ch % 2] = t

                k_lo, k_hi = ch * chunk, min((ch + 1) * chunk, s_len)
                for k in range(k_lo, k_hi):
                    for c in range(g):
                        cur = rings[c][(k // RING) % 2][:, (k % RING) * 32:
                                                        (k % RING) * 32 + 32]
                        # ---- renormalization event ----
                        if k >= EV0 and (k - EV0) % NK == 0:
                            e = (k - EV0) // NK
                            psc = pse_pool.tile([NGRP, 32], dt.float32,
                                                tag="psC", name=f"psC{c}_{k}")
                            nc.tensor.matmul(psc, ob_t, cur, start=True,
                                             stop=True)
                            rf = efxpool.tile([NGRP, 32], dt.float32, tag="rf", name=f"rf{c}_{k}")
                            nc.vector.reciprocal_approx_fast(out=rf, in_=psc)
                            rsb = rbufs[c][:, e * 32:(e + 1) * 32]
                            nc.vector.tensor_copy(rsb, rf)
                            psr = pse_pool.tile([P, 32], dt.float32, tag="psR", name=f"psR{c}_{k}")
                            nc.tensor.matmul(psr, oc_t, rsb, start=True,
                                             stop=True)
                            if k + LAG < s_len:
                                pend[c][k + LAG] = psr
                        # ---- ef slice (maybe renormalized) ----
                        efsl = chunk_tiles[c][(k // chunk) % 2][
                            :, (k % chunk) * 32:(k % chunk) * 32 + 32]
                        if k in pend[c]:
                            psr = pend[c].pop(k)
                            efx = efxpool.tile([P, 32], dt.bfloat16, tag="efx", name=f"efx{c}_{k}")
                            nc.vector.tensor_mul(efx, psr, efsl)
                            efsl = efx
                        # ---- main step: psum = Wblk^T @ q ; q' = psum * ef --
                        ps = psb_pool.tile([P, 32], dt.float32, tag=f"psB{c}", name=f"psB{c}_{k}")
                        nc.tensor.matmul(ps, w_t, cur, start=True, stop=True)
                        nxt = rings[c][((k + 1) // RING) % 2][
                            :, ((k + 1) % RING) * 32:((k + 1) % RING) * 32 + 32]
                        nc.vector.tensor_mul(nxt, ps, efsl)
                        # ---- snapshot dumps ----
                        if k + 1 == 3 * RING:
                            nc.sync.dma_start(out=snapA[c].ap(),
                                              in_=rings[c][0])
                        if k + 1 == 4 * RING:
                            nc.sync.dma_start(out=snapB[c].ap(),
                                              in_=rings[c][1])
                            nc.sync.dma_start(out=snapC[c].ap(),
                                              in_=rings[c][0][:, 0:32])

            for c in range(g):
                nc.sync.dma_start(out=rdump[c].ap(), in_=rbufs[c])

    nc.finalize()
    return nc


def _host_prep(feats, transition):
    """Returns per-core in_maps plus reconstruction metadata."""
    s_len, b_tot = feats.shape[0], feats.shape[1]
    n_cores = b_tot // BC
    P = 128 // G
    NGRP = P // 32
    c_pre = feats.max(axis=2)                                # (S, B)
    Ccum = np.vstack([np.zeros((1, b_tot), np.float64),
                      np.cumsum(c_pre.astype(np.float64), 0)])  # (S+1, B)
    ef = np.exp(feats - c_pre[:, :, None]).astype(BF)        # (S, B, T)

    W = np.exp(transition.astype(np.float64))                # [next, prev]
    lhs = W.T.astype(BF).astype(np.float32)                  # [prev, next]
    wblk = np.zeros((P, P), np.float32)
    for gi in range(NGRP):
        wblk[gi * 32:(gi + 1) * 32, gi * 32:(gi + 1) * 32] = lhs
    onesblk = np.zeros((P, NGRP), np.float32)
    for gi in range(NGRP):
        onesblk[gi * 32:(gi + 1) * 32, gi] = 1.0
    onesbc = np.zeros((NGRP, P), np.float32)
    for gi in range(NGRP):
        onesbc[gi, gi * 32:(gi + 1) * 32] = 1.0

    qinit = np.zeros((128, 32), np.float32)
    for gi in range(4):
        qinit[gi * 32 + START, :] = 1.0

    in_maps = []
    for core in range(n_cores):
        sl = slice(core * BC, (core + 1) * BC)
        A = ef[:, sl, :]                                     # (S, 128, 32)
        # brick+chunk layout: ef_d[g*32+tag, k*32+bi] = A[k, g*32+bi, tag]
        E = np.ascontiguousarray(
            A.reshape(s_len, 4, 32, 32).transpose(1, 3, 0, 2)
            .reshape(128, s_len * 32))
        in_maps.append({
            "ef": E.astype(BF),
            "qinit": qinit.astype(BF),
            "wblk": wblk.astype(BF),
            "onesblk": onesblk.astype(BF),
            "onesbc": onesbc.astype(BF),
        })
    return in_maps, Ccum


def _reconstruct(results, Ccum, transition, lengths, s_len=S):
    P = 128 // G
    NGRP = P // 32
    RING = s_len // 4
    n_cores = len(results)
    eT = np.exp(transition[END].astype(np.float64))          # (T,)
    n_ev = (s_len - EV0 - 1) // NK + 1
    k_apps = EV0 + NK * np.arange(n_ev) + LAG                # (E,)

    out = np.zeros(n_cores * BC, np.float64)
    for core in range(n_cores):
        res = results[core]
        for c in range(G):
            # snaps[:, j*32+bi] -> S_{2*RING+j}; stack A,B plus final C
            sA = res[f"snapsA_{c}"].astype(np.float32).reshape(NGRP, 32, RING, 32)
            sB = res[f"snapsB_{c}"].astype(np.float32).reshape(NGRP, 32, RING, 32)
            sC = res[f"snapsC_{c}"].astype(np.float32).reshape(NGRP, 32, 1, 32)
            snaps = np.concatenate([sA, sB, sC], axis=2)     # (g, tag, j, bi)
            rvals = res[f"rdump_{c}"].astype(np.float64).reshape(NGRP, n_ev, 32)
            lc = -np.log(np.maximum(rvals, 1e-300))
            for gi in range(NGRP):
                b0 = core * BC + c * P + gi * 32             # global b of bi=0
                bs = np.arange(b0, b0 + 32)
                L = lengths[bs]                              # (32,)
                qv = snaps[gi, :, L - 2 * RING, np.arange(32)]  # (32 bi, T)
                base = np.log(np.maximum(qv.astype(np.float64) @ eT, 1e-300))
                acc = Ccum[L, bs]
                inc = (k_apps[:, None] < L[None, :])         # (E, 32)
                acc = acc + (lc[gi] * inc).sum(axis=0)
                out[bs] = base + acc
    return out


_CACHED_NC = None
LAST_RESULTS = None         # BassKernelResults of the most recent run


def _enable_ldw_opt():
    """Rewrite walrus's --enable-ldw-opt=false to true (A/B experiment)."""
    import subprocess as _sp
    from concourse import bass_utils as _bu

    class _SpProxy:
        def __getattr__(self, a):
            return getattr(_sp, a)

        @staticmethod
        def check_call(argv, **kw):
            if isinstance(argv, list):
                argv = [a.replace("--enable-ldw-opt=false",
                                  "--enable-ldw-opt=true")
                        if isinstance(a, str) else a for a in argv]
            return _sp.check_call(argv, **kw)

    _bu.subprocess = _SpProxy()


def kernel(feats, mask, transition):
    global _CACHED_NC, LAST_RESULTS
    feats = np.asarray(feats, np.float32)
    mask = np.asarray(mask, np.float32)
    transition = np.asarray(transition, np.float32)
    lengths = mask.sum(axis=0).astype(np.int64)              # (B,)

    in_maps, Ccum = _host_prep(feats, transition)
    if _CACHED_NC is None:
        _CACHED_NC = build_program()
    if bool(int(os.environ.get("CRF_LDWOPT", "0"))):
        _enable_ldw_opt()
    trace = bool(int(os.environ.get("CRF_TRACE", "0")))
    if trace:
        try:  # supply the NTFF hook module this image's antenv lacks
            import types
            from trn_agent_boot.trn_boot import _ntff_profile_via_ctypes
            if "antenv.axon_hooks" not in sys.modules:
                m = types.ModuleType("antenv.axon_hooks")
                m._HOOK = None
                m.set_axon_ntff_profile_hook = lambda h: setattr(m, "_HOOK", h)
                m.get_axon_ntff_profile_hook = lambda: m._HOOK
                sys.modules["antenv.axon_hooks"] = m
            sys.modules["antenv.axon_hooks"].set_axon_ntff_profile_hook(
                _ntff_profile_via_ctypes("/opt/axon/libaxon_pjrt.so"))
        except Exception as e:  # profiling degrades, run still works
            print(f"ntff hook registration failed: {e}")
    res = run_bass_kernel_spmd(_CACHED_NC, in_maps, core_ids=list(range(NCORES)),
                               trace=trace)
    LAST_RESULTS = res
    out = _reconstruct(res.results, Ccum, transition, lengths)
    return out.astype(np.float32)


if __name__ == "__main__":
    feats = np.load("/tmp/in_feats.npy")
    mask = np.load("/tmp/in_mask.npy")
    trans = np.load("/tmp/in_transition.npy")
    got = kernel(feats, mask, trans)
    exp = np.load("/tmp/expected.npy")
    rel = np.abs(got - exp) / np.maximum(1.0, np.abs(exp))
    print("max rel:", rel.max(), "mean:", rel.mean())


# revision 23
# speedup vs baseline: 1.4570x; 1.4570x over previous
"""CRF forward (partition function) kernel for Trainium2, 8 NeuronCores.

Meet-in-the-middle formulation (exp space), data-parallel over batch:
  forward   F_{i+1} = ef_i * (W @ F_i),            i = 0..M-1   (alpha side)
  backward  G_t = W^T @ (ef_t * G_{t+1}) + 1[length==t] * exp(trans[END]),
run from both ends to the midpoint M = S/2 (lengths >= S/2, so the forward
half is mask-free); host combines out[b] = log(F_M . G_M) + accumulators.

W[next,prev] = exp(trans[next,prev]); ef is exp(feat - max_tag feat) (host
prescale, bookkept via cumsum); every NK steps the device renormalizes each
batch column by r ~ 1/colsum (computed on-device, applied to a later ef
slice, exact r values dumped for host compensation).

The backward injection rides inside the one matmul per step: the state is
augmented with 3 extra rows -- row 64 a self-perpetuating constant 1, rows
65/66 per-tag-group injection markers delivered via the ef stream (marker
row at time t = 1[length==t]); the stationary has columns that (a) copy the
constant row forward and (b) add exp(trans[END])[prev] * marker to each
group's state rows.  No extra instructions, no PSUM read-modify-write.

Layout per chain: 2 tag-groups of 32 tags stacked on partitions, 64 batch
elems on the free dim; one chain per direction (forward 64 partitions,
backward 67).  The serial critical path per chain step is the PE->DVE
semaphore round trip (~500ns); the two chains interleave on the engines.
"""

import os
import sys

import numpy as np
import ml_dtypes

if "/opt/trn_rl_repo" not in sys.path:
    sys.path.insert(0, "/opt/trn_rl_repo")

import concourse.bass as bass
import concourse.tile as tile
from concourse import bacc, mybir
from concourse.bass_utils import run_bass_kernel_spmd

BF = ml_dtypes.bfloat16
S, B, T = 1024, 1024, 32
START, END = T - 2, T - 1
NCORES = 8
BC = B // NCORES            # batch per core (128)
NK, EV0, LAG = 16, 4, 3     # renorm cadence / first event / apply lag
CHUNK = 128                 # steps per DMA chunk
P, NGRP, FD = 64, 2, 64     # partitions (tags), tag groups, batch free dim
PB = P + 3                  # backward partitions (+const row, +2 markers)

dt = mybir.dt


def build_program(s_len=S):
    """One SPMD program for all cores: forward + backward half-chains."""
    m = s_len // 2
    chunk = min(CHUNK, m)
    n_ev = (m - EV0 - 1) // NK + 1 if m > EV0 else 0

    nc = bacc.Bacc("TRN2", target_bir_lowering=False, num_devices=NCORES)

    efF_d = nc.dram_tensor("efF", [P, m * FD], dt.bfloat16, kind="ExternalInput")
    efB_d = nc.dram_tensor("efB", [PB, m * FD], dt.bfloat16, kind="ExternalInput")
    y0_d = nc.dram_tensor("y0", [PB, FD], dt.bfloat16, kind="ExternalInput")
    qiF_d = nc.dram_tensor("qinitF", [P, FD], dt.bfloat16, kind="ExternalInput")
    wF_d = nc.dram_tensor("wblkF", [P, P], dt.bfloat16, kind="ExternalInput")
    wB_d = nc.dram_tensor("wblkB", [PB, PB], dt.bfloat16, kind="ExternalInput")
    obF_d = nc.dram_tensor("onesblkF", [P, NGRP], dt.bfloat16, kind="ExternalInput")
    obB_d = nc.dram_tensor("onesblkB", [PB, NGRP], dt.bfloat16, kind="ExternalInput")
    oc_d = nc.dram_tensor("onesbc", [NGRP, P], dt.bfloat16, kind="ExternalInput")

    qF_o = nc.dram_tensor("qF", [P, FD], dt.bfloat16, kind="ExternalOutput")
    qB_o = nc.dram_tensor("qB", [P, FD], dt.bfloat16, kind="ExternalOutput")
    rdF_o = nc.dram_tensor("rdF", [NGRP, max(1, n_ev) * FD], dt.bfloat16,
                           kind="ExternalOutput")
    rdB_o = nc.dram_tensor("rdB", [NGRP, max(1, n_ev) * FD], dt.bfloat16,
                           kind="ExternalOutput")

    with tile.TileContext(nc) as tc:
        with (
            tc.tile_pool(name="singles", bufs=1) as singles,
            tc.tile_pool(name="efpool", bufs=2) as efpool,
            tc.tile_pool(name="small", bufs=2) as small,
            tc.tile_pool(name="ypool", bufs=3) as ypool,
            tc.tile_pool(name="psF", bufs=2, space="PSUM") as psf_pool,
            tc.tile_pool(name="psB", bufs=2, space="PSUM") as psb_pool,
            tc.tile_pool(name="psE", bufs=2, space="PSUM") as pse_pool,
        ):
            wF_t = singles.tile([P, P], dt.bfloat16, tag="wF", name="wF_t")
            wB_t = singles.tile([PB, PB], dt.bfloat16, tag="wB", name="wB_t")
            obF_t = singles.tile([P, NGRP], dt.bfloat16, tag="obF", name="obF_t")
            obB_t = singles.tile([PB, NGRP], dt.bfloat16, tag="obB", name="obB_t")
            oc_t = singles.tile([NGRP, P], dt.bfloat16, tag="oc", name="oc_t")
            for tl, dr in ((wF_t, wF_d), (wB_t, wB_d), (obF_t, obF_d),
                           (obB_t, obB_d), (oc_t, oc_d)):
                nc.sync.dma_start(out=tl, in_=dr.ap())

            stF = singles.tile([P, 2 * FD], dt.bfloat16, tag="stF", name="stF")
            rbF = singles.tile([NGRP, max(1, n_ev) * FD], dt.bfloat16,
                               tag="rbF", name="rbF")
            rbB = singles.tile([NGRP, max(1, n_ev) * FD], dt.bfloat16,
                               tag="rbB", name="rbB")
            nc.sync.dma_start(out=stF[:, 0:FD], in_=qiF_d.ap())

            y_cur = ypool.tile([PB, FD], dt.bfloat16, tag="y", name="y_0")
            nc.sync.dma_start(out=y_cur, in_=y0_d.ap())

            chF = [None, None]
            chB = [None, None]
            pendF, pendB = {}, {}

            def event(i, cur, ob_t, rbuf, pend, is_b):
                e = (i - EV0) // NK
                psc = pse_pool.tile([NGRP, FD], dt.float32, tag="psC",
                                    name=f"psC{int(is_b)}_{i}")
                nc.tensor.matmul(psc, ob_t, cur, start=True, stop=True)
                rf = small.tile([NGRP, FD], dt.float32, tag="rf",
                                name=f"rf{int(is_b)}_{i}")
                nc.vector.reciprocal_approx_fast(out=rf, in_=psc)
                rsb = rbuf[:, e * FD:(e + 1) * FD]
                nc.vector.tensor_copy(rsb, rf)
                psr = pse_pool.tile([P, FD], dt.float32, tag="psR",
                                    name=f"psR{int(is_b)}_{i}")
                nc.tensor.matmul(psr, oc_t, rsb, start=True, stop=True)
                if i + LAG < (m - 1 if is_b else m):
                    pend[i + LAG] = psr

            n_chunks = (m + chunk - 1) // chunk
            for ch in range(n_chunks):
                tF = efpool.tile([P, chunk * FD], dt.bfloat16, tag="efF",
                                 name=f"efF_{ch}")
                nc.sync.dma_start(
                    out=tF, in_=efF_d.ap()[:, ch * chunk * FD:
                                           (ch + 1) * chunk * FD])
                chF[ch % 2] = tF
                tB = efpool.tile([PB, chunk * FD], dt.bfloat16, tag="efB",
                                 name=f"efB_{ch}")
                nc.sync.dma_start(
                    out=tB, in_=efB_d.ap()[:, ch * chunk * FD:
                                           (ch + 1) * chunk * FD])
                chB[ch % 2] = tB

                for i in range(ch * chunk, min((ch + 1) * chunk, m)):
                    csl = slice((i % chunk) * FD, (i % chunk) * FD + FD)
                    # ---------------- forward chain, step i -----------------
                    curF = stF[:, (i % 2) * FD:(i % 2) * FD + FD]
                    if i >= EV0 and (i - EV0) % NK == 0:
                        event(i, curF, obF_t, rbF, pendF, is_b=False)
                    eslF = chF[(i // chunk) % 2][:, csl]
                    if i in pendF:
                        psr = pendF.pop(i)
                        efx = small.tile([P, FD], dt.bfloat16, tag="efxF",
                                         name=f"efxF_{i}")
                        nc.vector.tensor_mul(efx, psr, eslF)
                        eslF = efx
                    psf = psf_pool.tile([P, FD], dt.float32, tag="psf",
                                        name=f"psf_{i}")
                    nc.tensor.matmul(psf, wF_t, curF, start=True, stop=True)
                    nxtF = stF[:, ((i + 1) % 2) * FD:((i + 1) % 2) * FD + FD]
                    nc.vector.tensor_mul(nxtF, psf, eslF)

                    # ---------------- backward chain, step i ----------------
                    if i >= EV0 and (i - EV0) % NK == 0:
                        event(i, y_cur, obB_t, rbB, pendB, is_b=True)
                    psb = psb_pool.tile([PB, FD], dt.float32, tag="psb",
                                        name=f"psb_{i}")
                    nc.tensor.matmul(psb, wB_t, y_cur, start=True, stop=True)
                    if i < m - 1:
                        eslB = chB[(i // chunk) % 2][:, csl]
                        if i in pendB:
                            psr = pendB.pop(i)
                            efx = small.tile([PB, FD], dt.bfloat16, tag="efxB",
                                             name=f"efxB_{i}")
                            nc.vector.tensor_mul(efx[0:P, :], psr, eslB[0:P, :])
                            nc.vector.tensor_copy(efx[P:PB, :], eslB[P:PB, :])
                            eslB = efx
                        y_nxt = ypool.tile([PB, FD], dt.bfloat16, tag="y",
                                           name=f"y_{i + 1}")
                        nc.vector.tensor_mul(y_nxt, psb, eslB)
                        y_cur = y_nxt
                    else:
                        qB_t = singles.tile([P, FD], dt.bfloat16, tag="qBf",
                                            name="qB_t")
                        nc.vector.tensor_copy(qB_t, psb[0:P, :])

            nc.sync.dma_start(out=qF_o.ap(), in_=stF[:, (m % 2) * FD:
                                                     (m % 2) * FD + FD])
            nc.sync.dma_start(out=qB_o.ap(), in_=qB_t)
            nc.sync.dma_start(out=rdF_o.ap(), in_=rbF)
            nc.sync.dma_start(out=rdB_o.ap(), in_=rbB)

    nc.finalize()
    return nc


def _host_prep(feats, transition, lengths):
    """Per-core in_maps plus reconstruction metadata."""
    s_len, b_tot = feats.shape[0], feats.shape[1]
    n_cores = b_tot // BC
    m = s_len // 2
    c_pre = feats.max(axis=2)                                # (S, B)
    Ccum = np.vstack([np.zeros((1, b_tot), np.float64),
                      np.cumsum(c_pre.astype(np.float64), 0)])  # (S+1, B)
    ef = np.exp(feats - c_pre[:, :, None]).astype(BF)        # (S, B, T)

    W = np.exp(transition.astype(np.float64))                # [next, prev]
    lhsF = W.T.astype(BF).astype(np.float32)                 # [prev, next]
    lhsB = W.astype(BF).astype(np.float32)                   # [next, prev]
    eT = np.exp(transition[END].astype(np.float64))          # (T,)
    eTb = eT.astype(BF).astype(np.float32)

    wF = np.zeros((P, P), np.float32)
    wB = np.zeros((PB, PB), np.float32)
    for gi in range(NGRP):
        s32 = slice(gi * 32, (gi + 1) * 32)
        wF[s32, s32] = lhsF
        wB[s32, s32] = lhsB
        wB[P + 1 + gi, s32] = eTb                # marker row g -> inject eT
    wB[P, P:PB] = 1.0                            # const row perpetuates
    obF = np.zeros((P, NGRP), np.float32)
    obB = np.zeros((PB, NGRP), np.float32)
    onesbc = np.zeros((NGRP, P), np.float32)
    for gi in range(NGRP):
        obF[gi * 32:(gi + 1) * 32, gi] = 1.0
        obB[gi * 32:(gi + 1) * 32, gi] = 1.0
        onesbc[gi, gi * 32:(gi + 1) * 32] = 1.0
    obB[P, :] = 1.0                              # colsum += 1 (zero-col guard)

    qinitF = np.zeros((P, FD), np.float32)
    qinitF[START, :] = 1.0
    qinitF[32 + START, :] = 1.0

    in_maps = []
    for core in range(n_cores):
        sl = slice(core * BC, (core + 1) * BC)
        A = ef[:, sl, :]                                     # (S, 128, T)
        # brick: [g*32+tag, t, bi] = A[t, g*FD+bi, tag]
        E = (A.reshape(s_len, NGRP, FD, T).transpose(1, 3, 0, 2)
             .reshape(P, s_len, FD)).astype(np.float32)
        EF = np.ascontiguousarray(E[:, :m, :]).reshape(P, m * FD)
        Lc = lengths[sl].astype(int)                         # (128,)
        mark = np.zeros((NGRP, s_len + 1, FD), np.float32)   # [g, t, bi]
        for gi in range(NGRP):
            for bi in range(FD):
                mark[gi, Lc[gi * FD + bi], bi] = 1.0
        # backward stream col i <- t = s_len-2-i, rows: ef, 1, markers at t
        EB = np.zeros((PB, m, FD), np.float32)
        ts = s_len - 2 - np.arange(m)                        # (m,)
        EB[:P] = E[:, ts, :]
        EB[P] = 1.0
        EB[P + 1] = mark[0, ts, :]
        EB[P + 2] = mark[1, ts, :]
        EB = np.ascontiguousarray(EB).reshape(PB, m * FD)
        # y_0: rows = qinitB * ef_{S-1}, const 1, markers at t = S-1
        y0 = np.zeros((PB, FD), np.float32)
        for gi in range(NGRP):
            live = (Lc[gi * FD:(gi + 1) * FD] == s_len).astype(np.float32)
            y0[gi * 32:(gi + 1) * 32, :] = (
                eTb[:, None] * live[None, :] * E[gi * 32:(gi + 1) * 32,
                                                 s_len - 1, :])
        y0[P] = 1.0
        y0[P + 1] = mark[0, s_len - 1, :]
        y0[P + 2] = mark[1, s_len - 1, :]
        in_maps.append({
            "efF": EF.astype(BF),
            "efB": EB.astype(BF),
            "y0": y0.astype(BF),
            "qinitF": qinitF.astype(BF),
            "wblkF": wF.astype(BF),
            "wblkB": wB.astype(BF),
            "onesblkF": obF.astype(BF),
            "onesblkB": obB.astype(BF),
            "onesbc": onesbc.astype(BF),
        })
    return in_maps, Ccum


def _reconstruct(results, Ccum, transition, lengths, s_len=S):
    m = s_len // 2
    n_cores = len(results)
    n_ev = (m - EV0 - 1) // NK + 1 if m > EV0 else 0
    i_apps = EV0 + NK * np.arange(n_ev) + LAG                # (E,)

    out = np.zeros(n_cores * BC, np.float64)
    for core in range(n_cores):
        res = results[core]
        qF = res["qF"].astype(np.float64).reshape(NGRP, 32, FD)
        qB = res["qB"].astype(np.float64).reshape(NGRP, 32, FD)
        lcF = -np.log(np.maximum(
            res["rdF"].astype(np.float64).reshape(NGRP, n_ev, FD), 1e-300))
        lcB = -np.log(np.maximum(
            res["rdB"].astype(np.float64).reshape(NGRP, n_ev, FD), 1e-300))
        for gi in range(NGRP):
            bs = core * BC + gi * FD + np.arange(FD)
            L = lengths[bs]
            dot = (qF[gi] * qB[gi]).sum(axis=0)              # (FD,)
            base = np.log(np.maximum(dot, 1e-300))
            acc = Ccum[L, bs]
            acc = acc + lcF[gi].sum(axis=0)                  # all F events
            i_inj = (s_len - 1) - L                          # -1 when L==s_len
            incB = (i_apps[:, None] >= i_inj[None, :])       # (E, FD)
            acc = acc + (lcB[gi] * incB).sum(axis=0)
            out[bs] = base + acc
    return out


_CACHED_NC = None
LAST_RESULTS = None         # BassKernelResults of the most recent run


def kernel(feats, mask, transition):
    global _CACHED_NC, LAST_RESULTS
    feats = np.asarray(feats, np.float32)
    mask = np.asarray(mask, np.float32)
    transition = np.asarray(transition, np.float32)
    lengths = mask.sum(axis=0).astype(np.int64)              # (B,)

    in_maps, Ccum = _host_prep(feats, transition, lengths)
    if _CACHED_NC is None:
        _CACHED_NC = build_program()
    trace = bool(int(os.environ.get("CRF_TRACE", "0")))
    if trace:
        try:  # supply the NTFF hook module this image's antenv lacks
            import types
            from trn_agent_boot.trn_boot import _ntff_profile_via_ctypes
            if "antenv.axon_hooks" not in sys.modules:
                mm_ = types.ModuleType("antenv.axon_hooks")
                mm_._HOOK = None
                mm_.set_axon_ntff_profile_hook = lambda h: setattr(mm_, "_HOOK", h)
                mm_.get_axon_ntff_profile_hook = lambda: mm_._HOOK
                sys.modules["antenv.axon_hooks"] = mm_
            sys.modules["antenv.axon_hooks"].set_axon_ntff_profile_hook(
                _ntff_profile_via_ctypes("/opt/axon/libaxon_pjrt.so"))
        except Exception as e:  # profiling degrades, run still works
            print(f"ntff hook registration failed: {e}")
    res = run_bass_kernel_spmd(_CACHED_NC, in_maps, core_ids=list(range(NCORES)),
                               trace=trace)
    LAST_RESULTS = res
    out = _reconstruct(res.results, Ccum, transition, lengths)
    return out.astype(np.float32)


if __name__ == "__main__":
    feats = np.load("/tmp/in_feats.npy")
    mask = np.load("/tmp/in_mask.npy")
    trans = np.load("/tmp/in_transition.npy")
    got = kernel(feats, mask, trans)
    exp = np.load("/tmp/expected.npy")
    rel = np.abs(got - exp) / np.maximum(1.0, np.abs(exp))
    print("max rel:", rel.max(), "mean:", rel.mean())
